# revision 2
# baseline (speedup 1.0000x reference)
# GQA attention layer (B=1, S=2048, HID=2560, H=32, HKV=8, D=128) on 8 TRN2
# NeuronCores. Tensor-parallel over kv-head groups: core c owns kv head c and
# its 4 query heads (Wq/Wk/Wv row shards, Wo column shard). The o_proj
# partials are combined with an on-device ReduceScatter over the sequence
# axis (4 chunks, overlapped with compute); the host reassembles the
# sequence-sharded outputs.
#
# Per-core dataflow (all matmuls bf16 -> fp32 PSUM):
#   1. QKV projection from X^T tiles (s-major output layout), per-head
#      RMSNorm + RoPE on DVE, PE-transpose of Q/K into [d, s] layout.
#   2. Scores are computed transposed (S^T[k, q] = K Q^T) so that the
#      P^T @ V matmul needs no transpose of the 16.8M-element prob matrix.
#      exp() on the scalar engine (no max subtraction: |scores| is bounded),
#      softmax denominators via a ones-vector matmul on the PE (partition
#      sums), division folded into the attention-output eviction through a
#      K=1 broadcast matmul.
#   3. o_proj per 512-row chunk, ReduceScatter in bf16, host upcasts.
import sys

if "/opt/trn_rl_repo" not in sys.path:
    sys.path.insert(0, "/opt/trn_rl_repo")

import numpy as np
import ml_dtypes

import concourse.bacc as bacc
import concourse.mybir as mybir
import concourse.tile as tile
from concourse import bass_utils, masks

BF16 = mybir.dt.bfloat16
F32 = mybir.dt.float32

B, S, HID = 1, 2048, 2560
H, HKV, D = 32, 8, 128
G = H // HKV  # q heads per kv head (= per core)
NC = 8  # cores
DQ = G * D  # per-core q width (512)
EPS = 1e-6
SCALE = 1.0 / float(np.sqrt(D))

ST = 128          # s positions per compute tile
N_ST = S // ST    # 16
HC = HID // 128   # 20 contraction chunks
XL = 256          # s positions per X^T DMA load tile
N_XL = S // XL    # 8
QC = 512          # q positions per attention unit
N_QC = S // QC    # 4 (also the ReduceScatter chunk count)
N_KT = S // 128   # 16 k tiles per attention unit
NO = HID // 512   # 5 o_proj free-dim chunks

_NC_CACHE = None


def _build():
    nc = bacc.Bacc("TRN2", target_bir_lowering=False, debug=False, num_devices=NC)

    xt_d = nc.dram_tensor("xt", [N_XL, HC, 128, XL], BF16, kind="ExternalInput").ap()
    wq_d = nc.dram_tensor("wq", [HC, 128, DQ], BF16, kind="ExternalInput").ap()
    wkv_d = nc.dram_tensor("wkv", [HC, 128, 2 * D], BF16, kind="ExternalInput").ap()
    wo_d = nc.dram_tensor("wo", [G, 128, HID], BF16, kind="ExternalInput").ap()
    cwq_d = nc.dram_tensor("cwq", [N_ST, 128, D], F32, kind="ExternalInput").ap()
    swq_d = nc.dram_tensor("swq", [N_ST, 128, D], F32, kind="ExternalInput").ap()
    cwk_d = nc.dram_tensor("cwk", [N_ST, 128, D], F32, kind="ExternalInput").ap()
    swk_d = nc.dram_tensor("swk", [N_ST, 128, D], F32, kind="ExternalInput").ap()
    out_d = nc.dram_tensor("out", [S // NC, HID], BF16, kind="ExternalOutput").ap()

    with tile.TileContext(nc) as tc:
        with (
            tc.tile_pool(name="const", bufs=1) as cpool,
            tc.tile_pool(name="xt", bufs=2) as xt_pool,
            tc.tile_pool(name="cs", bufs=8) as cs_pool,
            tc.tile_pool(name="qw", bufs=6) as qw_pool,
            tc.tile_pool(name="kw", bufs=6) as kw_pool,
            tc.tile_pool(name="ro", bufs=2) as ro_pool,
            tc.tile_pool(name="sm", bufs=4) as sm_pool,
            tc.tile_pool(name="ep", bufs=2) as ep_pool,
            tc.tile_pool(name="ot", bufs=8) as ot_pool,
            tc.tile_pool(name="ob", bufs=2) as ob_pool,
            tc.tile_pool(name="psA", bufs=3, space="PSUM") as psA,
            tc.tile_pool(name="psB", bufs=2, space="PSUM") as psB,
            tc.tile_pool(name="psC", bufs=2, space="PSUM") as psC,
            tc.tile_pool(name="dram", bufs=1, space="DRAM") as dram,
        ):
            # ---- resident constants / weights ----
            ident = cpool.tile([128, 128], BF16, tag="ident")
            masks.make_identity(nc, ident[:])
            ones_k = cpool.tile([128, 1], BF16, tag="ones_k")
            nc.vector.memset(ones_k[:], 1.0)
            ones_m = cpool.tile([1, 128], F32, tag="ones_m")
            nc.vector.memset(ones_m[:], 1.0)

            wq_sb = cpool.tile([128, HC, DQ], BF16, tag="wq")
            nc.sync.dma_start(wq_sb[:], wq_d.rearrange("c p n -> p c n"))
            wkv_sb = cpool.tile([128, HC, 2 * D], BF16, tag="wkv")
            nc.sync.dma_start(wkv_sb[:], wkv_d.rearrange("c p n -> p c n"))
            wo_sb = cpool.tile([128, G, HID], BF16, tag="wo")
            nc.sync.dma_start(wo_sb[:], wo_d.rearrange("c p n -> p c n"))

            qt_sb = cpool.tile([128, G, S], BF16, tag="qt")   # Q^T  [d, h, s]
            kt_sb = cpool.tile([128, S], BF16, tag="kt")      # K^T  [d, s]
            v_sb = cpool.tile([128, N_KT, D], BF16, tag="v")  # V    [s%128, kt, d]

            # ================= phase 1: QKV + norm + rope + transpose ======
            xt_t = None
            for st in range(N_ST):
                if st % (XL // ST) == 0:
                    xt_t = xt_pool.tile([128, HC, XL], BF16, tag="xt")
                    nc.sync.dma_start(
                        xt_t[:], xt_d[st // (XL // ST)].rearrange("c p s -> p c s")
                    )
                soff = (st % (XL // ST)) * ST

                cwq_t = cs_pool.tile([128, D], F32, tag="cs")
                nc.sync.dma_start(cwq_t[:], cwq_d[st])
                swq_t = cs_pool.tile([128, D], F32, tag="cs")
                nc.sync.dma_start(swq_t[:], swq_d[st])
                cwk_t = cs_pool.tile([128, D], F32, tag="cs")
                nc.sync.dma_start(cwk_t[:], cwk_d[st])
                swk_t = cs_pool.tile([128, D], F32, tag="cs")
                nc.sync.dma_start(swk_t[:], swk_d[st])

                q_ps = psA.tile([128, DQ], F32, tag="a")
                kv_ps = psB.tile([128, 2 * D], F32, tag="b")
                for ch in range(HC):
                    lhs = xt_t[:, ch, soff : soff + ST]
                    nc.tensor.matmul(
                        q_ps[:], lhs, wq_sb[:, ch, :],
                        start=(ch == 0), stop=(ch == HC - 1),
                    )
                    nc.tensor.matmul(
                        kv_ps[:], lhs, wkv_sb[:, ch, :],
                        start=(ch == 0), stop=(ch == HC - 1),
                    )

                # evictions (scalar engine)
                q_sb = qw_pool.tile([128, DQ], F32, tag="qw")
                nc.scalar.copy(q_sb[:], q_ps[:])
                k_sb = kw_pool.tile([128, D], F32, tag="kw")
                nc.scalar.copy(k_sb[:], kv_ps[:, 0:D])
                nc.scalar.copy(v_sb[:, st, :], kv_ps[:, D : 2 * D])

                # ---- RMSNorm (per head) ----
                sq = qw_pool.tile([128, DQ], F32, tag="qw")
                nc.vector.tensor_mul(sq[:], q_sb[:], q_sb[:])
                ssq = sm_pool.tile([128, G], F32, tag="sm")
                nc.vector.tensor_reduce(
                    ssq[:], sq[:].rearrange("p (h d) -> p h d", d=D),
                    axis=mybir.AxisListType.X, op=mybir.AluOpType.add,
                )
                var = sm_pool.tile([128, G], F32, tag="sm")
                nc.vector.tensor_scalar(
                    var[:], ssq[:], 1.0 / D, EPS,
                    op0=mybir.AluOpType.mult, op1=mybir.AluOpType.add,
                )
                rt = sm_pool.tile([128, G], F32, tag="sm")
                nc.scalar.sqrt(rt[:], var[:])
                rq = sm_pool.tile([128, G], F32, tag="sm")
                nc.vector.reciprocal(rq[:], rt[:])

                ksq = kw_pool.tile([128, D], F32, tag="kw")
                nc.vector.tensor_mul(ksq[:], k_sb[:], k_sb[:])
                kssq = sm_pool.tile([128, G], F32, tag="sm")
                nc.vector.tensor_reduce(
                    kssq[:, 0:1], ksq[:].unsqueeze(1),
                    axis=mybir.AxisListType.X, op=mybir.AluOpType.add,
                )
                kvar = sm_pool.tile([128, G], F32, tag="sm")
                nc.vector.tensor_scalar(
                    kvar[:, 0:1], kssq[:, 0:1], 1.0 / D, EPS,
                    op0=mybir.AluOpType.mult, op1=mybir.AluOpType.add,
                )
                krt = sm_pool.tile([128, G], F32, tag="sm")
                nc.scalar.sqrt(krt[:, 0:1], kvar[:, 0:1])
                rk = sm_pool.tile([128, G], F32, tag="sm")
                nc.vector.reciprocal(rk[:, 0:1], krt[:, 0:1])

                # ---- normalize + rope (DVE) ----
                qn = qw_pool.tile([128, DQ], F32, tag="qw")
                qn3 = qn[:].rearrange("p (h d) -> p h d", d=D)
                nc.vector.tensor_tensor(
                    qn3, q_sb[:].rearrange("p (h d) -> p h d", d=D),
                    rq[:].unsqueeze(2).to_broadcast([128, G, D]),
                    op=mybir.AluOpType.mult,
                )
                t1 = qw_pool.tile([128, DQ], F32, tag="qw")
                t13 = t1[:].rearrange("p (h d) -> p h d", d=D)
                cwq3 = cwq_t[:].unsqueeze(1).to_broadcast([128, G, D])
                swq3 = swq_t[:].unsqueeze(1).to_broadcast([128, G, D])
                nc.vector.tensor_tensor(t13, qn3, cwq3, op=mybir.AluOpType.mult)
                u = qw_pool.tile([128, DQ], F32, tag="qw")
                u3 = u[:].rearrange("p (h d) -> p h d", d=D)
                hd = D // 2
                nc.vector.tensor_tensor(
                    u3[:, :, 0:hd], qn3[:, :, hd:D], swq3[:, :, 0:hd],
                    op=mybir.AluOpType.mult,
                )
                nc.vector.tensor_tensor(
                    u3[:, :, hd:D], qn3[:, :, 0:hd], swq3[:, :, hd:D],
                    op=mybir.AluOpType.mult,
                )
                qro = ro_pool.tile([128, DQ], BF16, tag="qro")
                qro3 = qro[:].rearrange("p (h d) -> p h d", d=D)
                nc.vector.tensor_sub(qro3[:, :, 0:hd], t13[:, :, 0:hd], u3[:, :, 0:hd])
                nc.vector.tensor_add(qro3[:, :, hd:D], t13[:, :, hd:D], u3[:, :, hd:D])

                kn = kw_pool.tile([128, D], F32, tag="kw")
                nc.vector.tensor_tensor(
                    kn[:], k_sb[:],
                    rk[:, 0:1].to_broadcast([128, D]),
                    op=mybir.AluOpType.mult,
                )
                kt1 = kw_pool.tile([128, D], F32, tag="kw")
                nc.vector.tensor_tensor(kt1[:], kn[:], cwk_t[:], op=mybir.AluOpType.mult)
                ku = kw_pool.tile([128, D], F32, tag="kw")
                nc.vector.tensor_tensor(
                    ku[:, 0:hd], kn[:, hd:D], swk_t[:, 0:hd], op=mybir.AluOpType.mult
                )
                nc.vector.tensor_tensor(
                    ku[:, hd:D], kn[:, 0:hd], swk_t[:, hd:D], op=mybir.AluOpType.mult
                )
                kro = ro_pool.tile([128, D], BF16, tag="kro")
                nc.vector.tensor_sub(kro[:, 0:hd], kt1[:, 0:hd], ku[:, 0:hd])
                nc.vector.tensor_add(kro[:, hd:D], kt1[:, hd:D], ku[:, hd:D])

                # ---- transpose Q heads + K into [d, s] ----
                for h in range(G):
                    tp = psC.tile([128, 128], BF16, tag="c")
                    nc.tensor.transpose(tp[:], qro[:, h * D : (h + 1) * D], ident[:])
                    nc.scalar.copy(qt_sb[:, h, st * ST : (st + 1) * ST], tp[:])
                tp = psC.tile([128, 128], BF16, tag="c")
                nc.tensor.transpose(tp[:], kro[:], ident[:])
                nc.scalar.copy(kt_sb[:, st * ST : (st + 1) * ST], tp[:])

            # ================= phase 2: attention + o_proj + RS ============
            for qc in range(N_QC):
                ot_tiles = []
                for h in range(G):
                    ep = ep_pool.tile([128, N_KT, QC], BF16, tag="ep")
                    for kt in range(N_KT):
                        s_ps = psA.tile([128, QC], F32, tag="a")
                        nc.tensor.matmul(
                            s_ps[:],
                            kt_sb[:, kt * 128 : (kt + 1) * 128],
                            qt_sb[:, h, qc * QC : (qc + 1) * QC],
                            start=True, stop=True,
                        )
                        nc.scalar.activation(
                            ep[:, kt, :], s_ps[:],
                            mybir.ActivationFunctionType.Exp, scale=SCALE,
                        )
                    sums_ps = psC.tile([1, QC], F32, tag="c")
                    pv_ps = psB.tile([128, QC], F32, tag="b")
                    for kt in range(N_KT):
                        nc.tensor.matmul(
                            sums_ps[:], ones_k[:], ep[:, kt, :],
                            start=(kt == 0), stop=(kt == N_KT - 1),
                        )
                        nc.tensor.matmul(
                            pv_ps[:], v_sb[:, kt, :], ep[:, kt, :],
                            start=(kt == 0), stop=(kt == N_KT - 1),
                        )
                    recip = sm_pool.tile([1, QC], F32, tag="rc")
                    nc.vector.reciprocal(recip[:], sums_ps[:])
                    bc_ps = psC.tile([128, QC], F32, tag="c")
                    nc.tensor.matmul(
                        bc_ps[:], ones_m[:], recip[:], start=True, stop=True
                    )
                    bcb = sm_pool.tile([128, QC], F32, tag="bcb")
                    nc.scalar.copy(bcb[:], bc_ps[:])
                    ot = ot_pool.tile([128, QC], BF16, tag="ot")
                    nc.vector.tensor_tensor(
                        ot[:], pv_ps[:], bcb[:], op=mybir.AluOpType.mult
                    )
                    ot_tiles.append(ot)

                # o_proj for this 512-row chunk
                rs_in = dram.tile([QC, HID], BF16, tag=f"rsin{qc}")
                rs_out = dram.tile([QC // NC, HID], BF16, tag=f"rsout{qc}")
                for sst in range(QC // ST):
                    ob = ob_pool.tile([128, HID], BF16, tag="ob")
                    for no in range(NO):
                        y_ps = psA.tile([128, 512], F32, tag="a")
                        for h in range(G):
                            nc.tensor.matmul(
                                y_ps[:],
                                ot_tiles[h][:, sst * ST : (sst + 1) * ST],
                                wo_sb[:, h, no * 512 : (no + 1) * 512],
                                start=(h == 0), stop=(h == G - 1),
                            )
                        nc.vector.tensor_copy(ob[:, no * 512 : (no + 1) * 512], y_ps[:])
                    nc.sync.dma_start(rs_in[sst * ST : (sst + 1) * ST, :], ob[:])

                nc.gpsimd.collective_compute(
                    "ReduceScatter",
                    mybir.AluOpType.add,
                    replica_groups=[list(range(NC))],
                    ins=[rs_in.opt()],
                    outs=[rs_out.opt()],
                )
                nc.sync.dma_start(
                    out_d[qc * (QC // NC) : (qc + 1) * (QC // NC), :], rs_out[:]
                )

    nc.compile()
    return nc


def _get_nc():
    global _NC_CACHE
    if _NC_CACHE is None:
        _NC_CACHE = _build()
    return _NC_CACHE


def make_in_maps(inputs):
    X = np.asarray(inputs["hidden_states"], dtype=np.float32).reshape(S, HID)
    freqs = np.asarray(inputs["freqs_cis"], dtype=np.float32)
    Wq = np.asarray(inputs["Wq"], dtype=np.float32)
    Wk = np.asarray(inputs["Wk"], dtype=np.float32)
    Wv = np.asarray(inputs["Wv"], dtype=np.float32)
    Wo = np.asarray(inputs["Wo"], dtype=np.float32)
    qw = np.asarray(inputs["q_norm_w"], dtype=np.float32)
    kw = np.asarray(inputs["k_norm_w"], dtype=np.float32)

    bf = ml_dtypes.bfloat16
    # X^T load tiles: (L, ch, p, s) = X[L*XL+s, ch*128+p]
    xt = np.ascontiguousarray(
        X.reshape(N_XL, XL, HC, 128).transpose(0, 2, 3, 1).astype(bf)
    )
    cos, sin = freqs[0], freqs[1]  # [S, D]
    cwq = np.ascontiguousarray((cos * qw[None, :]).reshape(N_ST, 128, D))
    swq = np.ascontiguousarray((sin * np.roll(qw, D // 2)[None, :]).reshape(N_ST, 128, D))
    cwk = np.ascontiguousarray((cos * kw[None, :]).reshape(N_ST, 128, D))
    swk = np.ascontiguousarray((sin * np.roll(kw, D // 2)[None, :]).reshape(N_ST, 128, D))

    in_maps = []
    for c in range(NC):
        wq_c = Wq[c * DQ : (c + 1) * DQ, :]  # [DQ, HID]
        wq_t = np.ascontiguousarray(wq_c.T.reshape(HC, 128, DQ).astype(bf))
        wk_c = Wk[c * D : (c + 1) * D, :]
        wv_c = Wv[c * D : (c + 1) * D, :]
        wkv_t = np.ascontiguousarray(
            np.concatenate([wk_c.T, wv_c.T], axis=1).reshape(HC, 128, 2 * D).astype(bf)
        )
        wo_c = Wo[:, c * DQ : (c + 1) * DQ]  # [HID, DQ]
        wo_t = np.ascontiguousarray(wo_c.T.reshape(G, 128, HID).astype(bf))
        in_maps.append(
            {
                "xt": xt,
                "wq": wq_t,
                "wkv": wkv_t,
                "wo": wo_t,
                "cwq": cwq,
                "swq": swq,
                "cwk": cwk,
                "swk": swk,
            }
        )
    return in_maps


def assemble(outs):
    # outs[c] is [S//NC, HID] bf16; chunk qc rows of core c cover global rows
    # [QC*qc + (QC//NC)*c, ...+QC//NC)
    y = np.empty((S, HID), dtype=np.float32)
    rows = QC // NC  # 64
    for qc in range(N_QC):
        for c in range(NC):
            y[QC * qc + rows * c : QC * qc + rows * (c + 1), :] = outs[c][
                rows * qc : rows * (qc + 1), :
            ].astype(np.float32)
    return y.reshape(B, S, HID)


def kernel(**inputs) -> np.ndarray:
    nc = _get_nc()
    in_maps = make_in_maps(inputs)
    res = bass_utils.run_bass_kernel_spmd(nc, in_maps, core_ids=list(range(NC)))
    return assemble([r["out"] for r in res.results])
